# revision 8
# baseline (speedup 1.0000x reference)
"""Causal self-attention (B=2, T=2048, D=1024, H=16, rope) on 8 Trainium2 cores.

Sharding: tensor-parallel over heads (2 heads/core): each core computes its
QKV projection columns, RoPE, causal attention, and a partial out-projection
(its rows of w_out); the host sums the 8 fp16 partials.

v4 (batch-pipelined):
 - Stage A: QKV projection for batch 0 (fp16, dense back-to-back matmuls).
 - Stage B: QKV for batch 1 interleaved with attention+out-proj of batch 0 —
   the projection keeps the PE saturated while the scalar engine works
   through softmax exps.
 - Stage C: attention + out-proj of batch 1.
 All matmul-accumulator tiles share one 3-slot PSUM tag (2 banks each), so
 the FIFO rotation paces the PE to the scalar engine without starving any
 stage; the softmax row tile (with the ones-row denominator) uses a 2-slot
 tag. Exp runs on ACT only (fp16, causally-trimmed columns on diagonal
 blocks); mask multiplies alternate DVE/GpSimd; RoPE runs on DVE; PSUM
 evacuations are split ACT/DVE. V' is re-blocked token-major with batched
 DMA transposes; x^T streams with 2-deep prefetch, split into half-chunk
 DMAs so matmuls start as soon as the first half lands.
"""

import sys

for _p in ("/opt/trn_rl_repo",):
    if _p not in sys.path:
        sys.path.insert(0, _p)

import numpy as np

B, T, D, H = 2, 2048, 1024, 16
DH = D // H  # 64
N_CORES = 8
HPC = H // N_CORES  # heads per core = 2
BT = B * T  # 4096
NK = D // 128   # 8 contraction chunks
NB = T // 128   # 16 key blocks per batch
NS = 4          # 512-token slices per batch
ROPE_BASE = 10000.0
EXP_BIAS = -4.0
VSCALE = 16.0   # w_v pre-scale, undone in w_out rows host-side

_CACHE = {}


def _host_consts():
    # RoPE tables, feature-major, two heads stacked: [128, T]
    inv_freq = 1.0 / (ROPE_BASE ** (np.arange(0, DH, 2, dtype=np.float32) / DH))
    t = np.arange(T, dtype=np.float32)
    freqs = np.outer(t, inv_freq)  # [T, 32]
    emb = np.concatenate([freqs, freqs], axis=-1)  # [T, 64]
    cosT = np.cos(emb).T.astype(np.float32)  # [64, T]
    sinT = np.sin(emb).T.astype(np.float32)
    # sign baked for the rotate-half term: rows 0:32 get -sin, rows 32:64 +sin
    sinS = np.concatenate([-sinT[:32], sinT[32:]], axis=0)
    cosb = np.concatenate([cosT, cosT], axis=0).astype(np.float16)
    sinb = np.concatenate([sinS, sinS], axis=0).astype(np.float16)
    # Causal masks for the 4 diagonal-block offsets o = 0,128,256,384,
    # concatenated along free dim: [128, 2048]
    p = np.arange(128)[:, None]
    f = np.arange(512)[None, :]
    mask = np.zeros((128, 4 * 512), dtype=np.float16)
    for tno in range(4):
        o = 128 * tno
        mask[:, tno * 512:(tno + 1) * 512] = (f >= o + p).astype(np.float16)
    return cosb, sinb, mask


def _build():
    from concourse import bacc
    import concourse.mybir as mybir
    import concourse.tile as tile

    F16 = mybir.dt.float16
    F32 = mybir.dt.float32
    AF = mybir.ActivationFunctionType

    nc = bacc.Bacc("TRN2", target_bir_lowering=False, debug=False,
                   num_devices=N_CORES)

    xt_d = nc.dram_tensor("xt", [D, BT], F16, kind="ExternalInput")
    wq_d = nc.dram_tensor("wq", [D, 128], F16, kind="ExternalInput")
    wk_d = nc.dram_tensor("wk", [D, 128], F16, kind="ExternalInput")
    wv_d = nc.dram_tensor("wv", [D, 128], F16, kind="ExternalInput")
    wo_d = nc.dram_tensor("wo16", [128, D], F16, kind="ExternalInput")
    cos_d = nc.dram_tensor("cosb", [128, T], F16, kind="ExternalInput")
    sin_d = nc.dram_tensor("sinb", [128, T], F16, kind="ExternalInput")
    mask_d = nc.dram_tensor("mask", [128, 2048], F16, kind="ExternalInput")
    out_d = nc.dram_tensor("outp", [D, BT], F16, kind="ExternalOutput")

    PIPE = 2  # exp->AV pipeline depth in groups

    with tile.TileContext(nc) as tc:
        with (
            tc.tile_pool(name="consts", bufs=1) as consts,
            tc.tile_pool(name="acts", bufs=1) as acts,
            tc.tile_pool(name="vpp", bufs=1) as vpp,
            tc.tile_pool(name="xtp", bufs=3) as xtp,
            tc.tile_pool(name="rope", bufs=3) as rope,
            tc.tile_pool(name="estp", bufs=4) as estp,
            tc.tile_pool(name="onp", bufs=2) as onp,
            tc.tile_pool(name="invp", bufs=2) as invp,
            tc.tile_pool(name="oevp", bufs=2) as oevp,
            tc.tile_pool(name="st_ps", bufs=3, space="PSUM") as st_ps,
            tc.tile_pool(name="u_ps", bufs=2, space="PSUM") as u_ps,
        ):
            # weights + first x slice first, rope tables next, rest later
            wq = consts.tile([128, NK, 128], F16)
            wk = consts.tile([128, NK, 128], F16)
            wv = consts.tile([128, NK, 128], F16)
            nc.sync.dma_start(out=wq, in_=wq_d[:, :].rearrange("(k p) f -> p k f", p=128))
            nc.sync.dma_start(out=wk, in_=wk_d[:, :].rearrange("(k p) f -> p k f", p=128))
            nc.sync.dma_start(out=wv, in_=wv_d[:, :].rearrange("(k p) f -> p k f", p=128))

            xt_r = xt_d[:, :].rearrange("(k p) t -> p k t", p=128)
            xtt_tiles = {}

            def xt_fetch(n):
                if n >= B * NS:
                    return
                tl = xtp.tile([128, NK, 512], F16, tag="xt", name=f"xt_pf{n}")
                xtt_tiles[n] = tl
                for hh in range(2):  # half-chunk split: matmuls start earlier
                    nc.sync.dma_start(
                        out=tl[:, 4 * hh:4 * hh + 4, :],
                        in_=xt_r[:, 4 * hh:4 * hh + 4, n * 512:(n + 1) * 512])

            xt_fetch(0)
            cosb = consts.tile([128, T], F16)
            sinb = consts.tile([128, T], F16)
            nc.sync.dma_start(out=cosb, in_=cos_d[:, :])
            nc.sync.dma_start(out=sinb, in_=sin_d[:, :])
            xt_fetch(1)
            wo = consts.tile([128, NK, 128], F16)
            nc.sync.dma_start(out=wo, in_=wo_d[:, :].rearrange("p (m f) -> p m f", m=NK))
            mask = consts.tile([128, 2048], F16)
            nc.sync.dma_start(out=mask, in_=mask_d[:, :])

            ones16 = consts.tile([128, NB], F16)
            nc.vector.memset(ones16, 1.0)
            ones64 = consts.tile([1, 64], F16)
            nc.vector.memset(ones64, 1.0)
            ebias = consts.tile([128, 1], F32)
            nc.vector.memset(ebias, EXP_BIAS)

            # preload the exp table while input DMAs stream
            dum_i = consts.tile([128, 1], F32)
            nc.vector.memset(dum_i, 0.0)
            dum_o = consts.tile([128, 1], F16)
            nc.scalar.activation(dum_o, dum_i, AF.Exp, scale=1.0, bias=ebias)

            qt = acts.tile([128, BT], F16)  # rows: [h0 d0..63 | h1 d0..63]
            kt = acts.tile([128, BT], F16)
            vt = acts.tile([128, BT], F16)

            vp16 = [[vpp.tile([128, NB, 80], F16, tag=f"vp16_{h}_{b}",
                              name=f"vp16_{h}_{b}") for h in range(HPC)]
                    for b in range(B)]
            for b in range(B):
                for h in range(HPC):
                    nc.vector.tensor_copy(vp16[b][h][:, :, 64], ones16)

            mflip = [0]

            def maskmul(est_ap, mask_ap):
                # alternate mask multiplies between GpSimd and DVE
                if mflip[0] % 2 == 0:
                    nc.gpsimd.tensor_mul(est_ap, est_ap, mask_ap)
                else:
                    nc.vector.tensor_mul(est_ap, est_ap, mask_ap)
                mflip[0] += 1

            def qkv_slice(n, evac_eng):
                b, s = divmod(n, NS)
                ts = slice(n * 512, (n + 1) * 512)
                cs = slice(s * 512, (s + 1) * 512)
                xtt = xtt_tiles.pop(n)
                xt_fetch(n + 2)

                raw = rope.tile([128, 2, 512], F16, tag="raw", name=f"raw{n}")
                swp = rope.tile([128, 2, 512], F16, tag="swp", name=f"swp{n}")
                rwc = rope.tile([128, 2, 512], F16, tag="rwc", name=f"rwc{n}")
                for pi, w in enumerate((wq, wk, wv)):
                    ps = st_ps.tile([128, 512], F32, tag="st",
                                    name=f"psqkv{n}_{pi}")
                    for g in range(NK):
                        nc.tensor.matmul(ps, w[:, g, :], xtt[:, g, :],
                                         start=(g == 0), stop=(g == NK - 1))
                    dst = raw[:, pi, :] if pi < 2 else vt[:, ts]
                    if evac_eng == "act":
                        nc.scalar.copy(dst, ps)
                    else:
                        nc.vector.tensor_copy(dst, ps)

                # RoPE (raw holds [q|k] for this slice), all on DVE
                for a2, b2 in ((0, 32), (32, 0), (64, 96), (96, 64)):
                    nc.sync.dma_start(out=swp[a2:a2 + 32], in_=raw[b2:b2 + 32])
                for pi, dst in ((0, qt), (1, kt)):
                    nc.vector.tensor_mul(rwc[:, pi, :], raw[:, pi, :], cosb[:, cs])
                    nc.vector.tensor_mul(swp[:, pi, :], swp[:, pi, :], sinb[:, cs])
                    nc.vector.tensor_add(dst[:, ts], rwc[:, pi, :], swp[:, pi, :])

                # V re-block token-major (batched DMA transpose, 4 blocks)
                for h in range(HPC):
                    nc.sync.dma_start_transpose(
                        out=vp16[b][h][:, 4 * s:4 * s + 4, 0:64],
                        in_=vt[h * 64:(h + 1) * 64, ts])

            def attention_j(b, s):
                t0 = b * T
                ts = slice((4 * b + s) * 512, (4 * b + s + 1) * 512)
                on_j = onp.tile([128, 512], F16, tag="on", name=f"on{b}_{s}")
                nblk = 4 * s + 4
                ngrp = nblk // 2
                qs = slice(t0 + s * 512, (t0 + s * 512) + 512)
                for h in range(HPC):
                    hp = h * 64
                    u = u_ps.tile([65, 512], F32, tag="u", name=f"u{b}_{s}_{h}")

                    def do_av(est0, g):
                        for t2 in range(2):
                            i = 2 * g + t2
                            o = max(0, 128 * (i - 4 * s))
                            nc.tensor.matmul(
                                u[:, o:512], vp16[b][h][:, i, 0:65],
                                est0[:, 512 * t2 + o:512 * (t2 + 1)],
                                start=(i == 0), stop=(i == nblk - 1),
                                skip_group_check=True)

                    pend = []
                    for g in range(ngrp):
                        st = st_ps.tile([128, 1024], F32, tag="st",
                                        name=f"st{b}_{s}_{h}_{g}")
                        for t2 in range(2):
                            i = 2 * g + t2
                            nc.tensor.matmul(
                                st[:, t2 * 512:(t2 + 1) * 512],
                                kt[hp:hp + 64, t0 + i * 128: t0 + (i + 1) * 128],
                                qt[hp:hp + 64, qs],
                                start=True, stop=True)
                        est = estp.tile([128, 1024], F16, tag="est16",
                                        name=f"est{b}_{s}_{h}_{g}")
                        if 2 * g + 1 < 4 * s:  # off-diagonal group
                            nc.scalar.activation(est, st, AF.Exp,
                                                 scale=float(DH) ** -0.5,
                                                 bias=ebias)
                        else:  # diagonal: trimmed exp + mask
                            for t2 in range(2):
                                i = 2 * g + t2
                                o = 128 * (i - 4 * s)
                                nc.scalar.activation(
                                    est[:, 512 * t2 + o:512 * (t2 + 1)],
                                    st[:, 512 * t2 + o:512 * (t2 + 1)],
                                    AF.Exp, scale=float(DH) ** -0.5,
                                    bias=ebias)
                                maskmul(
                                    est[:, 512 * t2 + o:512 * (t2 + 1)],
                                    mask[:, 512 * (i - 4 * s) + o:
                                         512 * (i - 4 * s + 1)])
                        pend.append((est, g))
                        if len(pend) > PIPE:
                            do_av(*pend.pop(0))
                    while pend:
                        do_av(*pend.pop(0))

                    # normalize: PE broadcast of r, fast reciprocal, multiply
                    rrow = invp.tile([1, 512], F16, tag="rrow",
                                     name=f"rr{b}_{s}_{h}")
                    nc.vector.tensor_copy(rrow, u[64:65, :])
                    bc = u_ps.tile([64, 512], F32, tag="u", name=f"bc{b}_{s}_{h}")
                    nc.tensor.matmul(bc, ones64, rrow, start=True, stop=True)
                    bci = invp.tile([64, 512], F32, tag="bci",
                                    name=f"bci{b}_{s}_{h}")
                    nc.vector.reciprocal_approx_fast(bci, bc)
                    nc.vector.tensor_mul(on_j[hp:hp + 64, :], u[0:64, :], bci)

                # out-projection for (b, j=s)
                ot = oevp.tile([128, NK, 512], F16, tag="ot", name=f"ot{b}_{s}")
                for m2 in range(NK // 2):
                    op = st_ps.tile([128, 1024], F32, tag="st",
                                    name=f"op{b}_{s}_{m2}")
                    for t2 in range(2):
                        m = 2 * m2 + t2
                        nc.tensor.matmul(op[:, t2 * 512:(t2 + 1) * 512],
                                         wo[:, m, :], on_j,
                                         start=True, stop=True)
                    dst = ot[:, 2 * m2:2 * m2 + 2, :].rearrange("p a f -> p (a f)")
                    if m2 % 2 == 0:
                        nc.vector.tensor_copy(dst, op)
                    else:
                        nc.scalar.copy(dst, op)
                # dispatch from ACT right after its m2=3 evacuation: no queue
                # wait, and Sync stays free for the x^T prefetch stream
                nc.scalar.dma_start(
                    out=out_d[:, :].rearrange("(m p) t -> p m t", p=128)[:, :, ts],
                    in_=ot)

            # Stage A: QKV batch 0
            for n in range(NS):
                qkv_slice(n, "act")
            # Stage B: QKV batch 1 interleaved with attention batch 0
            for s in range(NS):
                qkv_slice(NS + s, "dve")
                attention_j(0, s)
            # Stage C: attention batch 1
            for s in range(NS):
                attention_j(1, s)

    nc.compile()
    return nc


def _get_nc():
    if "nc" not in _CACHE:
        _CACHE["nc"] = _build()
    return _CACHE["nc"]


def _run(nc, in_maps, trace=False):
    from concourse.bass_utils import run_bass_kernel_spmd

    last = None
    for attempt in range(3):
        try:
            return run_bass_kernel_spmd(nc, in_maps,
                                        core_ids=list(range(N_CORES)),
                                        trace=trace)
        except Exception as e:  # transient device faults: retry
            last = e
            if "UNRECOVERABLE" not in str(e) and "UNAVAILABLE" not in str(e):
                raise
    raise last


def kernel(x, w_qkv, w_out, _trace=False):
    x = np.asarray(x, dtype=np.float32)
    w_qkv = np.asarray(w_qkv, dtype=np.float32)
    w_out = np.asarray(w_out, dtype=np.float32)

    xt = np.ascontiguousarray(x.reshape(BT, D).T).astype(np.float16)
    cosb, sinb, mask = _host_consts()

    in_maps = []
    for c in range(N_CORES):
        h0 = HPC * c
        cols = np.arange(h0 * DH, (h0 + HPC) * DH)
        in_maps.append({
            "xt": xt,
            "wq": np.ascontiguousarray(w_qkv[:, cols]).astype(np.float16),
            "wk": np.ascontiguousarray(w_qkv[:, D + cols]).astype(np.float16),
            "wv": (np.ascontiguousarray(w_qkv[:, 2 * D + cols])
                   * VSCALE).astype(np.float16),
            "wo16": (np.ascontiguousarray(w_out[cols, :]) / VSCALE).astype(np.float16),
            "cosb": cosb,
            "sinb": sinb,
            "mask": mask,
        })

    nc = _get_nc()
    res = _run(nc, in_maps, trace=_trace)
    acc = np.zeros((D, BT), dtype=np.float32)
    for c in range(N_CORES):
        acc += res.results[c]["outp"].astype(np.float32)
    out = acc.T.reshape(B, T, D)
    if _trace:
        return out, res
    return out


# revision 9
# speedup vs baseline: 1.0430x; 1.0430x over previous
"""Causal self-attention (B=2, T=2048, D=1024, H=16, rope) on 8 Trainium2 cores.

Sharding: tensor-parallel over heads (2 heads/core): each core computes its
QKV projection columns, RoPE, causal attention, and a partial out-projection
(its rows of w_out); the host sums the 8 fp16 partials.

v4 (batch-pipelined):
 - Stage A: QKV projection for batch 0 (fp16, dense back-to-back matmuls).
 - Stage B: QKV for batch 1 interleaved with attention+out-proj of batch 0 —
   the projection keeps the PE saturated while the scalar engine works
   through softmax exps.
 - Stage C: attention + out-proj of batch 1.
 All matmul-accumulator tiles share one 3-slot PSUM tag (2 banks each), so
 the FIFO rotation paces the PE to the scalar engine without starving any
 stage; the softmax row tile (with the ones-row denominator) uses a 2-slot
 tag. Exp runs on ACT only (fp16, causally-trimmed columns on diagonal
 blocks); mask multiplies alternate DVE/GpSimd; RoPE runs on DVE; PSUM
 evacuations are split ACT/DVE. V' is re-blocked token-major with batched
 DMA transposes; x^T streams with 2-deep prefetch, split into half-chunk
 DMAs so matmuls start as soon as the first half lands.
"""

import sys

for _p in ("/opt/trn_rl_repo",):
    if _p not in sys.path:
        sys.path.insert(0, _p)

import numpy as np

B, T, D, H = 2, 2048, 1024, 16
DH = D // H  # 64
N_CORES = 8
HPC = H // N_CORES  # heads per core = 2
BT = B * T  # 4096
NK = D // 128   # 8 contraction chunks
NB = T // 128   # 16 key blocks per batch
NS = 4          # 512-token slices per batch
ROPE_BASE = 10000.0
EXP_BIAS = -4.0
VSCALE = 16.0   # w_v pre-scale, undone in w_out rows host-side

_CACHE = {}


def _host_consts():
    # RoPE tables, feature-major, two heads stacked: [128, T]
    inv_freq = 1.0 / (ROPE_BASE ** (np.arange(0, DH, 2, dtype=np.float32) / DH))
    t = np.arange(T, dtype=np.float32)
    freqs = np.outer(t, inv_freq)  # [T, 32]
    emb = np.concatenate([freqs, freqs], axis=-1)  # [T, 64]
    cosT = np.cos(emb).T.astype(np.float32)  # [64, T]
    sinT = np.sin(emb).T.astype(np.float32)
    # sign baked for the rotate-half term: rows 0:32 get -sin, rows 32:64 +sin
    sinS = np.concatenate([-sinT[:32], sinT[32:]], axis=0)
    cosb = np.concatenate([cosT, cosT], axis=0).astype(np.float16)
    sinb = np.concatenate([sinS, sinS], axis=0).astype(np.float16)
    # Causal masks for the 4 diagonal-block offsets o = 0,128,256,384,
    # concatenated along free dim: [128, 2048]
    p = np.arange(128)[:, None]
    f = np.arange(512)[None, :]
    mask = np.zeros((128, 4 * 512), dtype=np.float16)
    for tno in range(4):
        o = 128 * tno
        mask[:, tno * 512:(tno + 1) * 512] = (f >= o + p).astype(np.float16)
    return cosb, sinb, mask


def _build():
    from concourse import bacc
    import concourse.mybir as mybir
    import concourse.tile as tile

    F8 = mybir.dt.float8e4
    F16 = mybir.dt.float16
    F32 = mybir.dt.float32
    AF = mybir.ActivationFunctionType
    DR = mybir.MatmulPerfMode.DoubleRow

    nc = bacc.Bacc("TRN2", target_bir_lowering=False, debug=False,
                   num_devices=N_CORES)

    xt_d = nc.dram_tensor("xt", [D, BT], F16, kind="ExternalInput")
    wq_d = nc.dram_tensor("wq", [D, 128], F16, kind="ExternalInput")
    wk_d = nc.dram_tensor("wk", [D, 128], F16, kind="ExternalInput")
    wv_d = nc.dram_tensor("wv", [D, 128], F16, kind="ExternalInput")
    wo_d = nc.dram_tensor("wo16", [128, D], F16, kind="ExternalInput")
    cos_d = nc.dram_tensor("cosb", [128, T], F16, kind="ExternalInput")
    sin_d = nc.dram_tensor("sinb", [128, T], F16, kind="ExternalInput")
    mask_d = nc.dram_tensor("mask", [128, 2048], F16, kind="ExternalInput")
    out_d = nc.dram_tensor("outp", [D, BT], F16, kind="ExternalOutput")

    PIPE = 2  # exp->AV pipeline depth in groups

    with tile.TileContext(nc) as tc:
        with (
            tc.tile_pool(name="consts", bufs=1) as consts,
            tc.tile_pool(name="acts", bufs=1) as acts,
            tc.tile_pool(name="vpp", bufs=1) as vpp,
            tc.tile_pool(name="xtp", bufs=4) as xtp,
            tc.tile_pool(name="rope", bufs=3) as rope,
            tc.tile_pool(name="estp", bufs=4) as estp,
            tc.tile_pool(name="onp", bufs=2) as onp,
            tc.tile_pool(name="invp", bufs=2) as invp,
            tc.tile_pool(name="oevp", bufs=2) as oevp,
            tc.tile_pool(name="st_ps", bufs=2, space="PSUM") as st_ps,
            tc.tile_pool(name="q1_ps", bufs=2, space="PSUM") as q1_ps,
            tc.tile_pool(name="u_ps", bufs=2, space="PSUM") as u_ps,
        ):
            # weights + first x slice first, rope tables next, rest later
            wq = consts.tile([128, NK, 128], F16)
            wk = consts.tile([128, NK, 128], F16)
            wv = consts.tile([128, NK, 128], F16)
            nc.sync.dma_start(out=wq, in_=wq_d[:, :].rearrange("(k p) f -> p k f", p=128))
            nc.sync.dma_start(out=wk, in_=wk_d[:, :].rearrange("(k p) f -> p k f", p=128))
            nc.sync.dma_start(out=wv, in_=wv_d[:, :].rearrange("(k p) f -> p k f", p=128))

            xt_r = xt_d[:, :].rearrange("(k p) t -> p k t", p=128)
            xtt_tiles = {}

            def xt_fetch(n):
                if n >= B * NS:
                    return
                tl = xtp.tile([128, NK, 512], F16, tag="xt", name=f"xt_pf{n}")
                xtt_tiles[n] = tl
                for hh in range(2):  # half-chunk split: matmuls start earlier
                    nc.sync.dma_start(
                        out=tl[:, 4 * hh:4 * hh + 4, :],
                        in_=xt_r[:, 4 * hh:4 * hh + 4, n * 512:(n + 1) * 512])

            xt_fetch(0)
            xt_fetch(1)
            cosb = consts.tile([128, T], F16)
            sinb = consts.tile([128, T], F16)
            nc.sync.dma_start(out=cosb, in_=cos_d[:, :])
            nc.sync.dma_start(out=sinb, in_=sin_d[:, :])
            xt_fetch(2)
            wo = consts.tile([128, NK, 128], F16)
            nc.sync.dma_start(out=wo, in_=wo_d[:, :].rearrange("p (m f) -> p m f", m=NK))
            mask = consts.tile([128, 2048], F16)
            nc.sync.dma_start(out=mask, in_=mask_d[:, :])

            ones16 = consts.tile([128, NB], F16)
            nc.vector.memset(ones16, 1.0)
            ones64 = consts.tile([1, 64], F16)
            nc.vector.memset(ones64, 1.0)
            ebias = consts.tile([128, 1], F32)
            nc.vector.memset(ebias, EXP_BIAS)

            # preload the exp table while input DMAs stream
            dum_i = consts.tile([128, 1], F32)
            nc.vector.memset(dum_i, 0.0)
            dum_o = consts.tile([128, 1], F16)
            nc.scalar.activation(dum_o, dum_i, AF.Exp, scale=1.0, bias=ebias)

            qt = acts.tile([128, BT], F16)  # rows: [h0 d0..63 | h1 d0..63]
            kt = acts.tile([128, BT], F16)
            vt = acts.tile([128, BT], F16)

            vp16 = [[vpp.tile([128, NB, 80], F16, tag=f"vp16_{h}_{b}",
                              name=f"vp16_{h}_{b}") for h in range(HPC)]
                    for b in range(B)]
            vp8 = [[vpp.tile([128, NB, 80], F8, tag=f"vp8_{h}_{b}",
                             name=f"vp8_{h}_{b}") for h in range(HPC)]
                   for b in range(B)]
            for b in range(B):
                for h in range(HPC):
                    nc.vector.tensor_copy(vp16[b][h][:, :, 64], ones16)

            mflip = [0]

            def maskmul(est_ap, mask_ap):
                # alternate mask multiplies between GpSimd and DVE
                if mflip[0] % 2 == 0:
                    nc.gpsimd.tensor_mul(est_ap, est_ap, mask_ap)
                else:
                    nc.vector.tensor_mul(est_ap, est_ap, mask_ap)
                mflip[0] += 1

            def qkv_slice(n, evac_eng):
                b, s = divmod(n, NS)
                ts = slice(n * 512, (n + 1) * 512)
                cs = slice(s * 512, (s + 1) * 512)
                xtt = xtt_tiles.pop(n)
                xt_fetch(n + 3)

                raw = rope.tile([128, 2, 512], F16, tag="raw", name=f"raw{n}")
                swp = rope.tile([128, 2, 512], F16, tag="swp", name=f"swp{n}")
                rwc = rope.tile([128, 2, 512], F16, tag="rwc", name=f"rwc{n}")
                for pi, w in enumerate((wq, wk, wv)):
                    ps = q1_ps.tile([128, 512], F32, tag="q1",
                                    name=f"psqkv{n}_{pi}")
                    for g in range(NK):
                        nc.tensor.matmul(ps, w[:, g, :], xtt[:, g, :],
                                         start=(g == 0), stop=(g == NK - 1))
                    dst = raw[:, pi, :] if pi < 2 else vt[:, ts]
                    if evac_eng == "act":
                        nc.scalar.copy(dst, ps)
                    else:
                        nc.vector.tensor_copy(dst, ps)

                # RoPE (raw holds [q|k] for this slice), all on DVE
                for a2, b2 in ((0, 32), (32, 0), (64, 96), (96, 64)):
                    nc.sync.dma_start(out=swp[a2:a2 + 32], in_=raw[b2:b2 + 32])
                for pi, dst in ((0, qt), (1, kt)):
                    nc.vector.tensor_mul(rwc[:, pi, :], raw[:, pi, :], cosb[:, cs])
                    nc.vector.tensor_mul(swp[:, pi, :], swp[:, pi, :], sinb[:, cs])
                    nc.vector.tensor_add(dst[:, ts], rwc[:, pi, :], swp[:, pi, :])

                # V re-block token-major (batched DMA transpose, 4 blocks)
                for h in range(HPC):
                    nc.sync.dma_start_transpose(
                        out=vp16[b][h][:, 4 * s:4 * s + 4, 0:64],
                        in_=vt[h * 64:(h + 1) * 64, ts])
                    nc.vector.tensor_copy(vp8[b][h][:, 4 * s:4 * s + 4, 0:65],
                                          vp16[b][h][:, 4 * s:4 * s + 4, 0:65])

            def attention_j(b, s):
                t0 = b * T
                ts = slice((4 * b + s) * 512, (4 * b + s + 1) * 512)
                on_j = onp.tile([128, 512], F16, tag="on", name=f"on{b}_{s}")
                nblk = 4 * s + 4
                ngrp = nblk // 2
                qs = slice(t0 + s * 512, (t0 + s * 512) + 512)
                for h in range(HPC):
                    hp = h * 64
                    u = u_ps.tile([65, 512], F32, tag="u", name=f"u{b}_{s}_{h}")

                    def do_av(est0, g):
                        if 2 * g + 1 < 4 * s:  # off-diagonal pair: fp8 DR
                            nc.tensor.matmul(
                                u, vp8[b][h][:, 2 * g:2 * g + 2, 0:65],
                                est0.rearrange("p (a f) -> p a f", a=2),
                                start=(g == 0), stop=False, perf_mode=DR,
                                skip_group_check=True)
                        else:  # diagonal: fp16, causally trimmed
                            for t2 in range(2):
                                i = 2 * g + t2
                                o = max(0, 128 * (i - 4 * s))
                                nc.tensor.matmul(
                                    u[:, o:512], vp16[b][h][:, i, 0:65],
                                    est0[:, 512 * t2 + o:512 * (t2 + 1)],
                                    start=(i == 0), stop=(i == nblk - 1),
                                    skip_group_check=True)

                    pend = []
                    for g in range(ngrp):
                        st = st_ps.tile([128, 1024], F32, tag="st",
                                        name=f"st{b}_{s}_{h}_{g}")
                        for t2 in range(2):
                            i = 2 * g + t2
                            nc.tensor.matmul(
                                st[:, t2 * 512:(t2 + 1) * 512],
                                kt[hp:hp + 64, t0 + i * 128: t0 + (i + 1) * 128],
                                qt[hp:hp + 64, qs],
                                start=True, stop=True)
                        if 2 * g + 1 < 4 * s:  # off-diagonal group: fp8
                            est = estp.tile([128, 1024], F8, tag="est8",
                                            name=f"est{b}_{s}_{h}_{g}")
                            nc.scalar.activation(est, st, AF.Exp,
                                                 scale=float(DH) ** -0.5,
                                                 bias=ebias)
                        else:  # diagonal: fp16, trimmed exp + mask
                            est = estp.tile([128, 1024], F16, tag="est16",
                                            name=f"est{b}_{s}_{h}_{g}")
                            for t2 in range(2):
                                i = 2 * g + t2
                                o = 128 * (i - 4 * s)
                                nc.scalar.activation(
                                    est[:, 512 * t2 + o:512 * (t2 + 1)],
                                    st[:, 512 * t2 + o:512 * (t2 + 1)],
                                    AF.Exp, scale=float(DH) ** -0.5,
                                    bias=ebias)
                                maskmul(
                                    est[:, 512 * t2 + o:512 * (t2 + 1)],
                                    mask[:, 512 * (i - 4 * s) + o:
                                         512 * (i - 4 * s + 1)])
                        pend.append((est, g))
                        if len(pend) > PIPE:
                            do_av(*pend.pop(0))
                    while pend:
                        do_av(*pend.pop(0))

                    # normalize: PE broadcast of r, fast reciprocal, multiply
                    rrow = invp.tile([1, 512], F16, tag="rrow",
                                     name=f"rr{b}_{s}_{h}")
                    nc.vector.tensor_copy(rrow, u[64:65, :])
                    bc = u_ps.tile([64, 512], F32, tag="u", name=f"bc{b}_{s}_{h}")
                    nc.tensor.matmul(bc, ones64, rrow, start=True, stop=True)
                    bci = invp.tile([64, 512], F32, tag="bci",
                                    name=f"bci{b}_{s}_{h}")
                    nc.vector.reciprocal_approx_fast(bci, bc)
                    nc.vector.tensor_mul(on_j[hp:hp + 64, :], u[0:64, :], bci)

                # out-projection for (b, j=s)
                ot = oevp.tile([128, NK, 512], F16, tag="ot", name=f"ot{b}_{s}")
                for m in range(NK):
                    op = u_ps.tile([128, 512], F32, tag="u",
                                   name=f"op{b}_{s}_{m}")
                    nc.tensor.matmul(op, wo[:, m, :], on_j, start=True,
                                     stop=True)
                    if m % 2 == 0:
                        nc.vector.tensor_copy(ot[:, m, :], op)
                    else:
                        nc.scalar.copy(ot[:, m, :], op)
                # dispatch from ACT right after its m2=3 evacuation: no queue
                # wait, and Sync stays free for the x^T prefetch stream
                nc.scalar.dma_start(
                    out=out_d[:, :].rearrange("(m p) t -> p m t", p=128)[:, :, ts],
                    in_=ot)

            # Stage A: QKV batch 0
            for n in range(NS):
                qkv_slice(n, "act")
            # Stage B: QKV batch 1 interleaved with attention batch 0
            for s in range(NS):
                qkv_slice(NS + s, "dve")
                attention_j(0, s)
            # Stage C: attention batch 1
            for s in range(NS):
                attention_j(1, s)

    nc.compile()
    return nc


def _get_nc():
    if "nc" not in _CACHE:
        _CACHE["nc"] = _build()
    return _CACHE["nc"]


def _run(nc, in_maps, trace=False):
    from concourse.bass_utils import run_bass_kernel_spmd

    last = None
    for attempt in range(3):
        try:
            return run_bass_kernel_spmd(nc, in_maps,
                                        core_ids=list(range(N_CORES)),
                                        trace=trace)
        except Exception as e:  # transient device faults: retry
            last = e
            if "UNRECOVERABLE" not in str(e) and "UNAVAILABLE" not in str(e):
                raise
    raise last


def kernel(x, w_qkv, w_out, _trace=False):
    x = np.asarray(x, dtype=np.float32)
    w_qkv = np.asarray(w_qkv, dtype=np.float32)
    w_out = np.asarray(w_out, dtype=np.float32)

    xt = np.ascontiguousarray(x.reshape(BT, D).T).astype(np.float16)
    cosb, sinb, mask = _host_consts()

    in_maps = []
    for c in range(N_CORES):
        h0 = HPC * c
        cols = np.arange(h0 * DH, (h0 + HPC) * DH)
        in_maps.append({
            "xt": xt,
            "wq": np.ascontiguousarray(w_qkv[:, cols]).astype(np.float16),
            "wk": np.ascontiguousarray(w_qkv[:, D + cols]).astype(np.float16),
            "wv": (np.ascontiguousarray(w_qkv[:, 2 * D + cols])
                   * VSCALE).astype(np.float16),
            "wo16": (np.ascontiguousarray(w_out[cols, :]) / VSCALE).astype(np.float16),
            "cosb": cosb,
            "sinb": sinb,
            "mask": mask,
        })

    nc = _get_nc()
    res = _run(nc, in_maps, trace=_trace)
    acc = np.zeros((D, BT), dtype=np.float32)
    for c in range(N_CORES):
        acc += res.results[c]["outp"].astype(np.float32)
    out = acc.T.reshape(B, T, D)
    if _trace:
        return out, res
    return out


# revision 10
# speedup vs baseline: 1.2274x; 1.1769x over previous
"""Causal self-attention (B=2, T=2048, D=1024, H=16, rope) on 8 Trainium2 cores.

Sharding: tensor-parallel over heads (2 heads/core): each core computes its
QKV projection columns, RoPE, causal attention, and a partial out-projection
(its rows of w_out); the host sums the 8 fp16 partials.

v7 (group-granular interleave):
 - Stage A: QKV projection of batch 0 (fp16, dense).
 - Stage B: attention of batch 0 with batch 1's QKV chunks and batch 0's
   out-projection chunks sprinkled between softmax groups, so the PE fills
   the bubbles of the exp-paced attention pipeline and the scalar engine
   never starves behind a block of projection work.
 - Stage C: attention of batch 1, out-projection interleaved the same way.
 Off-diagonal softmax tiles exp straight to fp8 and accumulate two key
 blocks per matmul via DoubleRow AV; diagonal tiles stay fp16 with causally
 trimmed score/exp columns and mask multiplies alternating DVE/GpSimd.
 All matmul accumulators share a 3-slot PSUM tag; the softmax row tile
 (ones-row denominator) and the PE-broadcast reciprocal share a 2-slot tag.
 PSUM evacuation: ACT in stage A (otherwise idle), DVE during B; the
 out-projection evacuations alternate DVE/ACT and output DMA is dispatched
 from ACT right after its own evacuation (keeps Sync free for x^T prefetch).
w_v is pre-scaled x16 (fp8 V' range), undone in w_out rows host-side.
"""

import itertools
import sys

for _p in ("/opt/trn_rl_repo",):
    if _p not in sys.path:
        sys.path.insert(0, _p)

import numpy as np
import ml_dtypes

B, T, D, H = 2, 2048, 1024, 16
DH = D // H  # 64
N_CORES = 8
HPC = H // N_CORES  # heads per core = 2
BT = B * T  # 4096
NK = D // 128   # 8 contraction chunks
NB = T // 128   # 16 key blocks per batch
NS = 4          # 512-token slices per batch
ROPE_BASE = 10000.0
EXP_BIAS = -4.0
VSCALE = 16.0   # w_v pre-scale (keeps 16*v inside fp8e4 max 240)
E4M3 = ml_dtypes.float8_e4m3

_CACHE = {}


def _host_consts():
    inv_freq = 1.0 / (ROPE_BASE ** (np.arange(0, DH, 2, dtype=np.float32) / DH))
    t = np.arange(T, dtype=np.float32)
    freqs = np.outer(t, inv_freq)  # [T, 32]
    emb = np.concatenate([freqs, freqs], axis=-1)  # [T, 64]
    cosT = np.cos(emb).T.astype(np.float32)  # [64, T]
    sinT = np.sin(emb).T.astype(np.float32)
    sinS = np.concatenate([-sinT[:32], sinT[32:]], axis=0)
    cosb = np.concatenate([cosT, cosT], axis=0).astype(np.float16)
    sinb = np.concatenate([sinS, sinS], axis=0).astype(np.float16)
    p = np.arange(128)[:, None]
    f = np.arange(512)[None, :]
    mask = np.zeros((128, 4 * 512), dtype=np.float16)
    for tno in range(4):
        o = 128 * tno
        mask[:, tno * 512:(tno + 1) * 512] = (f >= o + p).astype(np.float16)
    return cosb, sinb, mask


def _build():
    from concourse import bacc
    import concourse.mybir as mybir
    import concourse.tile as tile

    F8 = mybir.dt.float8e4
    F16 = mybir.dt.float16
    F32 = mybir.dt.float32
    AF = mybir.ActivationFunctionType
    DR = mybir.MatmulPerfMode.DoubleRow

    nc = bacc.Bacc("TRN2", target_bir_lowering=False, debug=False,
                   num_devices=N_CORES)

    xt_d = nc.dram_tensor("xt", [D, BT], F16, kind="ExternalInput")
    wq_d = nc.dram_tensor("wq", [D, 128], F16, kind="ExternalInput")
    wk_d = nc.dram_tensor("wk", [D, 128], F16, kind="ExternalInput")
    wv_d = nc.dram_tensor("wv", [D, 128], F16, kind="ExternalInput")
    wo_d = nc.dram_tensor("wo16", [128, D], F16, kind="ExternalInput")
    cos_d = nc.dram_tensor("cosb", [128, T], F16, kind="ExternalInput")
    sin_d = nc.dram_tensor("sinb", [128, T], F16, kind="ExternalInput")
    mask_d = nc.dram_tensor("mask", [128, 2048], F16, kind="ExternalInput")
    out_d = nc.dram_tensor("outp", [D, BT], F16, kind="ExternalOutput")

    PIPE = 2  # exp->AV pipeline depth in groups

    with tile.TileContext(nc) as tc:
        with (
            tc.tile_pool(name="consts", bufs=1) as consts,
            tc.tile_pool(name="acts", bufs=1) as acts,
            tc.tile_pool(name="vpp", bufs=1) as vpp,
            tc.tile_pool(name="xtp", bufs=4) as xtp,
            tc.tile_pool(name="rope", bufs=3) as rope,
            tc.tile_pool(name="estp", bufs=4) as estp,
            tc.tile_pool(name="onp", bufs=2) as onp,
            tc.tile_pool(name="invp", bufs=2) as invp,
            tc.tile_pool(name="oevp", bufs=2) as oevp,
            tc.tile_pool(name="st_ps", bufs=3, space="PSUM") as st_ps,
            tc.tile_pool(name="u_ps", bufs=2, space="PSUM") as u_ps,
        ):
            wq = consts.tile([128, NK, 128], F16)
            wk = consts.tile([128, NK, 128], F16)
            wv = consts.tile([128, NK, 128], F16)
            nc.sync.dma_start(out=wq, in_=wq_d[:, :].rearrange("(k p) f -> p k f", p=128))
            nc.sync.dma_start(out=wk, in_=wk_d[:, :].rearrange("(k p) f -> p k f", p=128))
            nc.sync.dma_start(out=wv, in_=wv_d[:, :].rearrange("(k p) f -> p k f", p=128))

            xt_r = xt_d[:, :].rearrange("(k p) t -> p k t", p=128)
            xtt_tiles = {}

            def xt_fetch(n):
                if n >= B * NS or n in xtt_tiles:
                    return
                tl = xtp.tile([128, NK, 512], F16, tag="xt", name=f"xt_pf{n}")
                xtt_tiles[n] = tl
                for hh in range(2):
                    nc.sync.dma_start(
                        out=tl[:, 4 * hh:4 * hh + 4, :],
                        in_=xt_r[:, 4 * hh:4 * hh + 4, n * 512:(n + 1) * 512])

            xt_fetch(0)
            xt_fetch(1)
            cosb = consts.tile([128, T], F16)
            sinb = consts.tile([128, T], F16)
            nc.sync.dma_start(out=cosb, in_=cos_d[:, :])
            nc.sync.dma_start(out=sinb, in_=sin_d[:, :])
            xt_fetch(2)
            wo = consts.tile([128, NK, 128], F16)
            nc.sync.dma_start(out=wo, in_=wo_d[:, :].rearrange("p (m f) -> p m f", m=NK))
            mask = consts.tile([128, 2048], F16)
            nc.sync.dma_start(out=mask, in_=mask_d[:, :])

            ones16 = consts.tile([128, NB], F16)
            nc.vector.memset(ones16, 1.0)
            ones64 = consts.tile([1, 64], F16)
            nc.vector.memset(ones64, 1.0)
            ebias = consts.tile([128, 1], F32)
            nc.vector.memset(ebias, EXP_BIAS)

            dum_i = consts.tile([128, 1], F32)
            nc.vector.memset(dum_i, 0.0)
            dum_o = consts.tile([128, 1], F16)
            nc.scalar.activation(dum_o, dum_i, AF.Exp, scale=1.0, bias=ebias)

            qt = acts.tile([128, BT], F16)
            kt = acts.tile([128, BT], F16)
            vt = acts.tile([128, BT], F16)

            vp16 = [[vpp.tile([128, NB, 80], F16, tag=f"vp16_{h}_{b}",
                              name=f"vp16_{h}_{b}") for h in range(HPC)]
                    for b in range(B)]
            vp8 = [[vpp.tile([128, NB, 80], F8, tag=f"vp8_{h}_{b}",
                             name=f"vp8_{h}_{b}") for h in range(HPC)]
                   for b in range(B)]
            for b in range(B):
                for h in range(HPC):
                    nc.vector.tensor_copy(vp16[b][h][:, :, 64], ones16)

            mflip = [0]

            def maskmul(est_ap, mask_ap):
                if mflip[0] % 2 == 0:
                    nc.gpsimd.tensor_mul(est_ap, est_ap, mask_ap)
                else:
                    nc.vector.tensor_mul(est_ap, est_ap, mask_ap)
                mflip[0] += 1

            def qkv_chunks(n, evac_eng):
                """Yield thunks: 3 projection chunks, rope, transposes."""
                b, s = divmod(n, NS)
                ts = slice(n * 512, (n + 1) * 512)
                cs = slice(s * 512, (s + 1) * 512)
                state = {}

                def setup():
                    state["xtt"] = xtt_tiles.pop(n)
                    xt_fetch(n + 3)
                    state["raw"] = rope.tile([128, 2, 512], F16, tag="raw",
                                             name=f"raw{n}")
                    state["swp"] = rope.tile([128, 2, 512], F16, tag="swp",
                                             name=f"swp{n}")
                    state["rwc"] = rope.tile([128, 2, 512], F16, tag="rwc",
                                             name=f"rwc{n}")

                def proj(pi, w):
                    def run():
                        if pi == 0:
                            setup()
                        ps = st_ps.tile([128, 512], F32, tag="st",
                                        name=f"psqkv{n}_{pi}")
                        for g in range(NK):
                            nc.tensor.matmul(ps, w[:, g, :], state["xtt"][:, g, :],
                                             start=(g == 0), stop=(g == NK - 1))
                        dst = state["raw"][:, pi, :] if pi < 2 else vt[:, ts]
                        if evac_eng == "act":
                            nc.scalar.copy(dst, ps)
                        else:
                            nc.vector.tensor_copy(dst, ps)
                    return run

                def rope_chunk():
                    raw, swp, rwc = state["raw"], state["swp"], state["rwc"]
                    for a2, b2 in ((0, 32), (32, 0), (64, 96), (96, 64)):
                        nc.sync.dma_start(out=swp[a2:a2 + 32], in_=raw[b2:b2 + 32])
                    for pi, dst in ((0, qt), (1, kt)):
                        nc.vector.tensor_mul(rwc[:, pi, :], raw[:, pi, :], cosb[:, cs])
                        nc.vector.tensor_mul(swp[:, pi, :], swp[:, pi, :], sinb[:, cs])
                        nc.vector.tensor_add(dst[:, ts], rwc[:, pi, :], swp[:, pi, :])

                def tp_chunk():
                    for h in range(HPC):
                        nc.sync.dma_start_transpose(
                            out=vp16[b][h][:, 4 * s:4 * s + 4, 0:64],
                            in_=vt[h * 64:(h + 1) * 64, ts])
                        nc.vector.tensor_copy(
                            vp8[b][h][:, 4 * s:4 * s + 4, 0:65],
                            vp16[b][h][:, 4 * s:4 * s + 4, 0:65])

                yield proj(0, wq)
                yield proj(1, wk)
                yield proj(2, wv)
                yield rope_chunk
                yield tp_chunk

            def outproj_chunks(b, s, on_j):
                ts = slice((4 * b + s) * 512, (4 * b + s + 1) * 512)
                ot = oevp.tile([128, NK, 512], F16, tag="ot", name=f"ot{b}_{s}")

                def op_chunk(m2):
                    def run():
                        op = st_ps.tile([128, 1024], F32, tag="st",
                                        name=f"op{b}_{s}_{m2}")
                        for t2 in range(2):
                            m = 2 * m2 + t2
                            nc.tensor.matmul(op[:, t2 * 512:(t2 + 1) * 512],
                                             wo[:, m, :], on_j,
                                             start=True, stop=True)
                        dst = ot[:, 2 * m2:2 * m2 + 2, :].rearrange(
                            "p a f -> p (a f)")
                        if m2 % 2 == 0:
                            nc.vector.tensor_copy(dst, op)
                        else:
                            nc.scalar.copy(dst, op)
                    return run

                def dma_chunk():
                    nc.scalar.dma_start(
                        out=out_d[:, :].rearrange(
                            "(m p) t -> p m t", p=128)[:, :, ts],
                        in_=ot)

                for m2 in range(NK // 2):
                    yield op_chunk(m2)
                yield dma_chunk

            def attention_j(b, s, filler):
                t0 = b * T
                on_j = onp.tile([128, 512], F16, tag="on", name=f"on{b}_{s}")
                nblk = 4 * s + 4
                ngrp = nblk // 2
                q0 = t0 + s * 512

                def fill():
                    for th in filler:
                        th()
                        return

                for h in range(HPC):
                    hp = h * 64
                    u = u_ps.tile([65, 512], F32, tag="u", name=f"u{b}_{s}_{h}")

                    def do_av(est0, g):
                        if 2 * g + 1 < 4 * s:  # off-diagonal pair: fp8 DR
                            nc.tensor.matmul(
                                u, vp8[b][h][:, 2 * g:2 * g + 2, 0:65],
                                est0.rearrange("p (a f) -> p a f", a=2),
                                start=(g == 0), stop=False, perf_mode=DR,
                                skip_group_check=True)
                        else:  # diagonal: fp16, causally trimmed
                            for t2 in range(2):
                                i = 2 * g + t2
                                o = max(0, 128 * (i - 4 * s))
                                nc.tensor.matmul(
                                    u[:, o:512], vp16[b][h][:, i, 0:65],
                                    est0[:, 512 * t2 + o:512 * (t2 + 1)],
                                    start=(i == 0), stop=(i == nblk - 1),
                                    skip_group_check=True)

                    pend = []
                    for g in range(ngrp):
                        diag = not (2 * g + 1 < 4 * s)
                        st = st_ps.tile([128, 1024], F32, tag="st",
                                        name=f"st{b}_{s}_{h}_{g}")
                        for t2 in range(2):
                            i = 2 * g + t2
                            o = max(0, 128 * (i - 4 * s)) if diag else 0
                            nc.tensor.matmul(
                                st[:, t2 * 512 + o:(t2 + 1) * 512],
                                kt[hp:hp + 64, t0 + i * 128: t0 + (i + 1) * 128],
                                qt[hp:hp + 64, q0 + o:q0 + 512],
                                start=True, stop=True)
                        if not diag:  # off-diagonal group: fp8 est
                            est = estp.tile([128, 1024], F8, tag="est8",
                                            name=f"est{b}_{s}_{h}_{g}")
                            nc.scalar.activation(est, st, AF.Exp,
                                                 scale=float(DH) ** -0.5,
                                                 bias=ebias)
                        else:  # diagonal: fp16, trimmed exp + mask
                            est = estp.tile([128, 1024], F16, tag="est16",
                                            name=f"est{b}_{s}_{h}_{g}")
                            for t2 in range(2):
                                i = 2 * g + t2
                                o = 128 * (i - 4 * s)
                                nc.scalar.activation(
                                    est[:, 512 * t2 + o:512 * (t2 + 1)],
                                    st[:, 512 * t2 + o:512 * (t2 + 1)],
                                    AF.Exp, scale=float(DH) ** -0.5,
                                    bias=ebias)
                                maskmul(
                                    est[:, 512 * t2 + o:512 * (t2 + 1)],
                                    mask[:, 512 * (i - 4 * s) + o:
                                         512 * (i - 4 * s + 1)])
                        pend.append((est, g))
                        if len(pend) > PIPE:
                            do_av(*pend.pop(0))
                        fill()  # sprinkle one interleaved chunk per group
                    while pend:
                        do_av(*pend.pop(0))

                    # normalize
                    rrow = invp.tile([1, 512], F16, tag="rrow",
                                     name=f"rr{b}_{s}_{h}")
                    nc.vector.tensor_copy(rrow, u[64:65, :])
                    bc = u_ps.tile([64, 512], F32, tag="u", name=f"bc{b}_{s}_{h}")
                    nc.tensor.matmul(bc, ones64, rrow, start=True, stop=True)
                    bci = invp.tile([64, 512], F32, tag="bci",
                                    name=f"bci{b}_{s}_{h}")
                    nc.vector.reciprocal_approx_fast(bci, bc)
                    nc.vector.tensor_mul(on_j[hp:hp + 64, :], u[0:64, :], bci)
                    fill()
                return on_j

            def drain(filler):
                for th in filler:
                    th()

            # Stage A: QKV batch 0, dense
            for n in range(NS):
                drain(qkv_chunks(n, "act"))
            # Stage B: attention b0 with qkv(b1) + outproj(b0) interleaved
            pending = iter(())
            for s in range(NS):
                filler = itertools.chain(pending, qkv_chunks(NS + s, "dve"))
                on_j = attention_j(0, s, filler)
                drain(filler)
                pending = outproj_chunks(0, s, on_j)
            # Stage C: attention b1, outproj interleaved
            for s in range(NS):
                filler = pending
                on_j = attention_j(1, s, filler)
                drain(filler)
                pending = outproj_chunks(1, s, on_j)
            drain(pending)

    nc.compile()
    return nc


def _get_nc():
    if "nc" not in _CACHE:
        _CACHE["nc"] = _build()
    return _CACHE["nc"]


def _run(nc, in_maps, trace=False):
    from concourse.bass_utils import run_bass_kernel_spmd

    last = None
    for attempt in range(3):
        try:
            return run_bass_kernel_spmd(nc, in_maps,
                                        core_ids=list(range(N_CORES)),
                                        trace=trace)
        except Exception as e:  # transient device faults: retry
            last = e
            if "UNRECOVERABLE" not in str(e) and "UNAVAILABLE" not in str(e):
                raise
    raise last


def kernel(x, w_qkv, w_out, _trace=False):
    x = np.asarray(x, dtype=np.float32)
    w_qkv = np.asarray(w_qkv, dtype=np.float32)
    w_out = np.asarray(w_out, dtype=np.float32)

    xt = np.ascontiguousarray(x.reshape(BT, D).T).astype(np.float16)
    cosb, sinb, mask = _host_consts()

    in_maps = []
    for c in range(N_CORES):
        h0 = HPC * c
        cols = np.arange(h0 * DH, (h0 + HPC) * DH)
        in_maps.append({
            "xt": xt,
            "wq": np.ascontiguousarray(w_qkv[:, cols]).astype(np.float16),
            "wk": np.ascontiguousarray(w_qkv[:, D + cols]).astype(np.float16),
            "wv": (np.ascontiguousarray(w_qkv[:, 2 * D + cols])
                   * VSCALE).astype(np.float16),
            "wo16": (np.ascontiguousarray(w_out[cols, :]) / VSCALE).astype(np.float16),
            "cosb": cosb,
            "sinb": sinb,
            "mask": mask,
        })

    nc = _get_nc()
    res = _run(nc, in_maps, trace=_trace)
    acc = np.zeros((D, BT), dtype=np.float32)
    for c in range(N_CORES):
        acc += res.results[c]["outp"].astype(np.float32)
    out = acc.T.reshape(B, T, D)
    if _trace:
        return out, res
    return out
